# revision 25
# baseline (speedup 1.0000x reference)
"""FLAME layer on 8 Trainium2 NeuronCores (Bass/Tile).

Strategy (vertex-parallel):
  * V=5023 padded to 5120 and split 640 vertices/core; every core handles
    the full batch B=2048 for its vertex slice. This replicates only the
    small per-batch operands (betas, pose features, joint transforms)
    while the big model tensor (shapedirs, 24MB) is sliced 1/8 per core.
  * Host (O(B) + O(model) prep): rodrigues, forward-kinematic chain,
    A_rel; J is linear in betas (J = J0 + betas @ SJ with SJ precomputed
    from J_regressor x shapedirs), so no device dependency on v_shaped.
    Host also pre-transposes operands so the device does zero transposes.
  * Device (all O(B*V) work), per 128-vertex chunk in [v(part), b(free)]
    layout:
      1. vposed_c = sdt_aug_c.T @ betas_aug  (PE; K=437 = 400 blendshape
         rows + v_template x ones row + 36 posedirs rows; the bf16 hi/lo
         3-pass is stacked into K=1311 rows [hi;hi;lo] x [hi;lo;hi] for
         ~1e-6 accuracy at bf16 speed)
      2. T = blend of per-joint transforms (PE): per output plane one
         K=15 bf16 matmul whose rows stack the hi/lo split 3-pass
         ([w_hi;w_hi;w_lo] x [A_hi;A_lo;A_hi]), row-tiled 4x via
         tile_position so the 4 planes of one c run concurrently
      3. verts_c = T_c0*vp_x + T_c1*vp_y + T_c2*vp_z + T_c3  (DVE)
      4. DMA out [3, 640, 2048]; host reassembles [B, V, 3].
"""

import os
from contextlib import ExitStack

import ml_dtypes
import numpy as np

import bass_rust
import concourse.bass as bass
import concourse.mybir as mybir
import concourse.tile as tile_mod
from concourse.bass_utils import run_bass_kernel_spmd

# ---------------------------------------------------------------- constants
B = 2048
V = 5023
VP = 5120            # padded vertex count (8 cores x 640)
NVC = VP // 8        # vertices per core
NCHUNK = NVC // 128  # 128-vertex chunks per core (5)
NJ = 5
NCORES = 8
BH = 1024            # batch half (free-dim tile for T/apply)
KB = 437             # vposed contraction rows: 400 betas + 1 ones + 36 pose
KS = 3 * KB          # stacked hi/lo rows: [hi;hi;lo] x [hi;lo;hi]
KCH = [(k, min(128, KS - k)) for k in range(0, KS, 128)]
NS = 512             # matmul free-dim slice (one PSUM bank)
PARENTS = np.array([-1, 0, 1, 1, 1])

# T-blend: per c one round of 4 concurrent row-tiled K=15 matmuls,
# group g handles plane (c, n=g).

BF16 = ml_dtypes.bfloat16

# ------------------------------------------------- walrus multi-wait patch
# This walrus build accepts only ONE sem-wait per instruction (CTRL and
# LW queue structs alike), but Tile freely assigns several. Split the
# surplus waits onto same-engine NOPs emitted immediately before the
# instruction — the engine stalls on each NOP's wait first, so the
# gating semantics are identical.


def _patched_commit_instruction(self, inst, lazy_reg_writes=True):
    si = inst.sync_info
    if si is not None and len(si.on_wait) > 1:
        waits = list(si.on_wait)
        inst.sync_info = bass_rust.SyncInfo(
            on_update=list(si.on_update), on_wait=waits[:1]
        )
        for w in waits[1:]:
            nop = mybir.InstNoOp(
                name=self.nc.get_next_instruction_name(),
                engine=inst.engine,
                ins=[],
                outs=[],
                bass_nofuse=True,
                sync_info=bass_rust.SyncInfo(on_update=[], on_wait=[w]),
            )
            _orig_commit_instruction(self, nop, lazy_reg_writes=False)
    return _orig_commit_instruction(self, inst, lazy_reg_writes)


def _split_inst_waits(nc, inst):
    si = inst.ins.sync_info
    if si is None:
        return
    waits = list(si.on_wait)
    if len(waits) <= 1:
        return
    inst.ins.sync_info = bass_rust.SyncInfo(
        on_update=list(si.on_update), on_wait=waits[:1]
    )
    for i in range(1, len(waits)):
        nop = nc.sync.nop(nofuse=True, hint="drain_wait_split")
        nop.ins.sync_info = bass_rust.SyncInfo(on_update=[], on_wait=[waits[i]])


def _patched_drain_and_barrier(self, tick_clock, wait_clock):
    nc = self.nc
    drain_inst = nc.sync.drain()
    wait_clock.add_sem_waits(
        drain_inst.ins, tile_mod.ScopedClock({None: tick_clock.global_clock})
    )
    _split_inst_waits(nc, drain_inst)
    nc.all_engine_barrier()
    assert self.sems is not None
    popped = nc._tile_sem_poison_stack.pop()
    assert popped is self._sem_poison
    nc.clear_and_free_semaphores(list(self.sems.allocated().values()))
    nc.all_engine_barrier()


_orig_commit_instruction = tile_mod.TileContext._commit_instruction
if getattr(tile_mod.TileContext, "_flame_wait_patch", False) is False:
    tile_mod.TileContext._commit_instruction = _patched_commit_instruction
    tile_mod.TileContext._drain_and_barrier = _patched_drain_and_barrier
    tile_mod.TileContext._flame_wait_patch = True

# ----------------------------------------------------------- host-side math


def _rodrigues(r):
    angle = np.linalg.norm(r, axis=-1, keepdims=True) + 1e-8
    axis = r / angle
    x, y, z = axis[..., 0], axis[..., 1], axis[..., 2]
    zero = np.zeros_like(x)
    K = np.stack([zero, -z, y, z, zero, -x, -y, x, zero], axis=-1)
    K = K.reshape(r.shape[:-1] + (3, 3))
    s = np.sin(angle)[..., None]
    c = np.cos(angle)[..., None]
    return np.eye(3, dtype=r.dtype) + s * K + (1.0 - c) * (K @ K)


def _split_hi_lo(x):
    hi = x.astype(np.float32).astype(BF16)
    lo = (x.astype(np.float32) - hi.astype(np.float32)).astype(BF16)
    return np.ascontiguousarray(hi), np.ascontiguousarray(lo)


def _plane_cn(cn):
    return cn // 4, cn % 4


def _host_batch_prep(shape, expression, rotation, neck, jaw, eyeballs,
                     v_template, shapedirs, J_regressor):
    f64 = np.float64
    b = shape.shape[0]
    betas = np.concatenate([shape, expression], axis=1).astype(f64)
    full_pose = np.concatenate([rotation, neck, jaw, eyeballs], axis=1).astype(f64)

    jr = J_regressor.astype(f64)
    J0 = jr @ v_template.astype(f64)                                   # [5,3]
    SJ = np.einsum('jv,vcl->ljc', jr, shapedirs.astype(f64)).reshape(400, NJ * 3)
    J = (J0.reshape(-1) + betas @ SJ).reshape(b, NJ, 3)

    rot_mats = _rodrigues(full_pose.reshape(b, NJ, 3))
    pose_feature = (rot_mats[:, 1:] - np.eye(3, dtype=f64)).reshape(b, 36)

    rel_joints = np.concatenate([J[:, :1], J[:, 1:] - J[:, PARENTS[1:]]], axis=1)
    T_local = np.zeros((b, NJ, 4, 4), dtype=f64)
    T_local[:, :, :3, :3] = rot_mats
    T_local[:, :, :3, 3] = rel_joints
    T_local[:, :, 3, 3] = 1.0
    chain = [T_local[:, 0]]
    for j in range(1, NJ):
        chain.append(chain[PARENTS[j]] @ T_local[:, j])
    A = np.stack(chain, axis=1)

    j_hom = np.concatenate([J, np.zeros_like(J[..., :1])], axis=-1)
    t_corr = np.einsum('bjmn,bjn->bjm', A, j_hom)
    A_rel = (A - np.concatenate(
        [np.zeros_like(A[..., :3]), t_corr[..., None]], axis=-1)
    ).astype(np.float32)                                               # [B,5,4,4]

    # betas_aug [437, B]: rows 0-399 betas.T, 400 ones, 401-436 pose_feature.T
    betas_aug = np.empty((KB, b), dtype=np.float32)
    betas_aug[:400] = betas.T
    betas_aug[400] = 1.0
    betas_aug[401:] = pose_feature.T
    bt_hi, bt_lo = _split_hi_lo(betas_aug)
    bt_stk = np.concatenate(
        [bt_hi, bt_lo, bt_hi], axis=0)                     # [KS, B] bf16

    # stacked row-tiled A_rel operand: artc[c] rows 32g+u pair with
    # wrt_stk rows so one K=15 matmul per plane (c, n=g) computes
    # w_hi*A_hi + w_hi*A_lo + w_lo*A_hi
    a_hi, a_lo = _split_hi_lo(A_rel)
    a_hi = a_hi.astype(np.float32)
    a_lo = a_lo.astype(np.float32)
    artc = np.zeros((3, 128, b), dtype=np.float32)
    for c in range(3):
        for n in range(4):
            for j in range(NJ):
                artc[c, 32 * n + j] = a_hi[:, j, c, n]
                artc[c, 32 * n + 5 + j] = a_lo[:, j, c, n]
                artc[c, 32 * n + 10 + j] = a_hi[:, j, c, n]
    artc = artc.astype(BF16)
    return bt_stk, artc


def _host_model_prep(v_template, shapedirs, posedirs, lbs_weights):
    # sdt_aug [3, 437, VP] matching betas_aug rows
    sdt = np.zeros((3, KB, VP), dtype=np.float32)
    sdt[:, :400, :V] = shapedirs.transpose(1, 2, 0)
    sdt[:, 400, :V] = v_template.T
    sdt[:, 401:, :V] = posedirs.reshape(36, V, 3).transpose(2, 0, 1)
    sdt_hi, sdt_lo = _split_hi_lo(sdt)
    sdt_stk = np.concatenate(
        [sdt_hi, sdt_hi, sdt_lo], axis=1)                  # [3, KS, VP] bf16

    # wrt_stk [128, VP]: rows 32g+u = [w_hi; w_hi; w_lo][u] (u<15),
    # identical for every group g
    w_hi, w_lo = _split_hi_lo(lbs_weights)
    w_hi = w_hi.astype(np.float32)
    w_lo = w_lo.astype(np.float32)
    wrt = np.zeros((128, VP), dtype=np.float32)
    for g in range(4):
        for j in range(NJ):
            wrt[32 * g + j, :V] = w_hi[:, j]
            wrt[32 * g + 5 + j, :V] = w_hi[:, j]
            wrt[32 * g + 10 + j, :V] = w_lo[:, j]
    wrt_stk = wrt.astype(BF16)
    return sdt_stk, wrt_stk

# ------------------------------------------------------------ device kernel


def _build_device_program():
    nc = bass.Bass("TRN2", target_bir_lowering=False, debug=False)
    f32 = mybir.dt.float32
    f32r = mybir.dt.float32r
    bf16 = mybir.dt.bfloat16

    sdt = nc.dram_tensor("sdt", [3, KS, NVC], bf16, kind="ExternalInput").ap()
    wrt = nc.dram_tensor("wrt", [128, NVC], bf16, kind="ExternalInput").ap()
    bt = nc.dram_tensor("bt", [KS, B], bf16, kind="ExternalInput").ap()
    artc = nc.dram_tensor("artc", [3, 128, B], bf16, kind="ExternalInput").ap()
    out = nc.dram_tensor("out", [3, NVC, B], f32, kind="ExternalOutput").ap()

    with tile_mod.TileContext(nc) as tc, ExitStack() as ctx:
        cpool = ctx.enter_context(tc.tile_pool(name="const", bufs=1))
        spool = ctx.enter_context(tc.tile_pool(name="stream", bufs=2))
        vpool = ctx.enter_context(tc.tile_pool(name="vposed", bufs=2))
        tpool = ctx.enter_context(tc.tile_pool(name="tblend", bufs=2))
        apool = ctx.enter_context(tc.tile_pool(name="apply", bufs=2))
        ps_v = ctx.enter_context(tc.tile_pool(name="psv", bufs=2, space="PSUM"))
        ps_t = ctx.enter_context(tc.tile_pool(name="pst", bufs=4, space="PSUM"))

        # resident operands (scalar HWDGE queue so the per-chunk streaming
        # DMAs on the sync queue aren't stuck behind them at startup)
        btt = []
        for ki, (k0, kn) in enumerate(KCH):
            th = cpool.tile([kn, B], bf16, tag=f"bt{ki}", name=f"bt{ki}")
            nc.scalar.dma_start(th[:], bt[k0:k0 + kn, :])
            btt.append(th)
        wrtt = cpool.tile([128, NVC], bf16, tag="wrtt")
        nc.scalar.dma_start(wrtt[:], wrt[:, :])
        artt = []
        for c in range(3):
            t = cpool.tile([128, B], bf16, tag=f"artc{c}", name=f"artc{c}")
            nc.scalar.dma_start(t[:], artc[c, :, :])
            artt.append(t)

        for k in range(NCHUNK):
            vs = slice(k * 128, (k + 1) * 128)

            # stream this chunk's vposed lhsT tiles (stacked bf16)
            st = []
            for c in range(3):
                row = []
                for ki, (k0, kn) in enumerate(KCH):
                    t = spool.tile([kn, 128], bf16, tag=f"st{c}_{ki}",
                                   name=f"st{c}_{ki}")
                    nc.sync.dma_start(t[:], sdt[c, k0:k0 + kn, vs])
                    row.append(t)
                st.append(row)

            # 1) vposed planes [128, B]: stacked hi/lo, K=1311 in 11 chunks
            nkc = len(KCH)
            vp = []
            for c in range(3):
                dst = vpool.tile([128, B], mybir.dt.float32, tag=f"vp{c}")
                for nsp in range(B // 1024):
                    acc = ps_v.tile([128, 1024], mybir.dt.float32, tag="psv")
                    for half in range(2):
                        ps = slice(half * NS, (half + 1) * NS)
                        ns = 2 * nsp + half
                        bs = slice(ns * NS, (ns + 1) * NS)
                        for ki in range(nkc):
                            nc.tensor.matmul(
                                acc[:, ps], lhsT=st[c][ki][:],
                                rhs=btt[ki][:, bs],
                                start=(ki == 0), stop=(ki == nkc - 1))
                    nc.scalar.copy(out=dst[:, nsp * 1024:(nsp + 1) * 1024],
                                   in_=acc[:])
                vp.append(dst)

            # 2+3) per (batch-half, c): row-tiled K=15 stacked-bf16 T blend
            # for the 4 planes of this c, then the affine apply on DVE
            for h in range(B // BH):
                hb = slice(h * BH, (h + 1) * BH)
                for c in range(3):
                    tt = tpool.tile([128, 4 * BH], mybir.dt.float32, tag="tt")
                    for ns in range(BH // NS):
                        src = slice(h * BH + ns * NS, h * BH + (ns + 1) * NS)
                        accs = [ps_t.tile([128, NS], mybir.dt.float32,
                                          name=f"tacc{g_}", tag="pst")
                                for g_ in range(4)]
                        for n in range(4):
                            p0 = 32 * n
                            nc.tensor.matmul(
                                accs[n][:],
                                lhsT=wrtt[p0:p0 + 15, vs],
                                rhs=artt[c][p0:p0 + 15, src],
                                start=True, stop=True,
                                tile_position=(p0, 0))
                        for n in range(4):
                            nc.scalar.copy(
                                out=tt[:, n * BH + ns * NS:n * BH + (ns + 1) * NS],
                                in_=accs[n][:])

                    def tsl(n):
                        return tt[:, n * BH:(n + 1) * BH]
                    ma = apool.tile([128, BH], mybir.dt.float32, tag="ma")
                    mb = apool.tile([128, BH], mybir.dt.float32, tag="mb")
                    mc = apool.tile([128, BH], mybir.dt.float32, tag="mc")
                    nc.vector.tensor_mul(ma[:], tsl(0), vp[0][:, hb])
                    nc.gpsimd.tensor_mul(mc[:], tsl(2), vp[2][:, hb])
                    nc.vector.tensor_mul(mb[:], tsl(1), vp[1][:, hb])
                    nc.vector.tensor_add(ma[:], ma[:], mb[:])
                    nc.gpsimd.tensor_add(mc[:], mc[:], tsl(3))
                    nc.vector.tensor_add(ma[:], ma[:], mc[:])
                    nc.sync.dma_start(out[c, vs, hb], ma[:])
    return nc


_NC_CACHE = {}


def _get_nc():
    if "nc" not in _NC_CACHE:
        _NC_CACHE["nc"] = _build_device_program()
    return _NC_CACHE["nc"]

# ---------------------------------------------------------------- entry


def build_in_maps(shape, expression, rotation, neck, jaw, eyeballs,
                  v_template, shapedirs, posedirs, J_regressor, lbs_weights):
    bt_stk, artc = _host_batch_prep(
        shape, expression, rotation, neck, jaw, eyeballs,
        v_template, shapedirs, J_regressor)
    sdt_stk, wrt_stk = _host_model_prep(
        v_template, shapedirs, posedirs, lbs_weights)

    in_maps = []
    for i in range(NCORES):
        v0, v1 = i * NVC, (i + 1) * NVC
        in_maps.append({
            "sdt": np.ascontiguousarray(sdt_stk[:, :, v0:v1]),
            "wrt": np.ascontiguousarray(wrt_stk[:, v0:v1]),
            "bt": bt_stk,
            "artc": artc,
        })
    return in_maps


def kernel(shape, expression, rotation, neck, jaw, eyeballs,
           v_template, shapedirs, posedirs, J_regressor, lbs_weights):
    in_maps = build_in_maps(shape, expression, rotation, neck, jaw, eyeballs,
                            v_template, shapedirs, posedirs, J_regressor,
                            lbs_weights)
    nc = _get_nc()
    res = run_bass_kernel_spmd(nc, in_maps, core_ids=list(range(NCORES)))

    full = np.concatenate([res.results[i]["out"] for i in range(NCORES)], axis=1)
    verts = np.ascontiguousarray(full[:, :V, :].transpose(2, 1, 0))
    return verts.astype(np.float32)


# revision 28
# speedup vs baseline: 1.1467x; 1.1467x over previous
"""FLAME layer on 8 Trainium2 NeuronCores (Bass/Tile).

Strategy (vertex-parallel):
  * V=5023 padded to 5120 and split 640 vertices/core; every core handles
    the full batch B=2048 for its vertex slice. This replicates only the
    small per-batch operands (betas, pose features, joint transforms)
    while the big model tensor (shapedirs, 24MB) is sliced 1/8 per core.
  * Host (O(B) + O(model) prep): rodrigues, forward-kinematic chain,
    A_rel; J is linear in betas (J = J0 + betas @ SJ with SJ precomputed
    from J_regressor x shapedirs), so no device dependency on v_shaped.
    Host also pre-transposes operands so the device does zero transposes.
  * Device (all O(B*V) work), per 128-vertex chunk in [v(part), b(free)]
    layout:
      1. vposed_c = sdt_aug_c.T @ betas_aug  (PE; K=437 = 400 blendshape
         rows + v_template x ones row + 36 posedirs rows; the bf16 hi/lo
         3-pass is stacked into K=1311 rows [hi;hi;lo] x [hi;lo;hi] for
         ~1e-6 accuracy at bf16 speed)
      2. T = blend of per-joint transforms (PE): per output plane one
         K=15 bf16 matmul whose rows stack the hi/lo split 3-pass
         ([w_hi;w_hi;w_lo] x [A_hi;A_lo;A_hi]), row-tiled 4x via
         tile_position so the 4 planes of one c run concurrently
      3. verts_c = T_c0*vp_x + T_c1*vp_y + T_c2*vp_z + T_c3  (DVE)
      4. DMA out [3, 640, 2048]; host reassembles [B, V, 3].
"""

import os
from contextlib import ExitStack

import ml_dtypes
import numpy as np

import bass_rust
import concourse.bass as bass
import concourse.mybir as mybir
import concourse.tile as tile_mod
from concourse.bass_utils import run_bass_kernel_spmd

# ---------------------------------------------------------------- constants
B = 2048
V = 5023
VP = 5120            # padded vertex count (8 cores x 640)
NVC = VP // 8        # vertices per core
NCHUNK = NVC // 128  # 128-vertex chunks per core (5)
NJ = 5
NCORES = 8
BH = 1024            # batch half (free-dim tile for T/apply)
KB = 437             # vposed contraction rows: 400 betas + 1 ones + 36 pose
KS = 3 * KB          # stacked hi/lo rows: [hi;hi;lo] x [hi;lo;hi]
KCH = [(k, min(128, KS - k)) for k in range(0, KS, 128)]
NS = 512             # matmul free-dim slice (one PSUM bank)
PARENTS = np.array([-1, 0, 1, 1, 1])

# T-blend: per c one round of 4 concurrent row-tiled K=15 matmuls,
# group g handles plane (c, n=g).

BF16 = ml_dtypes.bfloat16

# ------------------------------------------------- walrus multi-wait patch
# This walrus build accepts only ONE sem-wait per instruction (CTRL and
# LW queue structs alike), but Tile freely assigns several. Split the
# surplus waits onto same-engine NOPs emitted immediately before the
# instruction — the engine stalls on each NOP's wait first, so the
# gating semantics are identical.


def _patched_commit_instruction(self, inst, lazy_reg_writes=True):
    si = inst.sync_info
    if si is not None and len(si.on_wait) > 1:
        waits = list(si.on_wait)
        inst.sync_info = bass_rust.SyncInfo(
            on_update=list(si.on_update), on_wait=waits[:1]
        )
        for w in waits[1:]:
            nop = mybir.InstNoOp(
                name=self.nc.get_next_instruction_name(),
                engine=inst.engine,
                ins=[],
                outs=[],
                bass_nofuse=True,
                sync_info=bass_rust.SyncInfo(on_update=[], on_wait=[w]),
            )
            _orig_commit_instruction(self, nop, lazy_reg_writes=False)
    return _orig_commit_instruction(self, inst, lazy_reg_writes)


def _split_inst_waits(nc, inst):
    si = inst.ins.sync_info
    if si is None:
        return
    waits = list(si.on_wait)
    if len(waits) <= 1:
        return
    inst.ins.sync_info = bass_rust.SyncInfo(
        on_update=list(si.on_update), on_wait=waits[:1]
    )
    for i in range(1, len(waits)):
        nop = nc.sync.nop(nofuse=True, hint="drain_wait_split")
        nop.ins.sync_info = bass_rust.SyncInfo(on_update=[], on_wait=[waits[i]])


def _patched_drain_and_barrier(self, tick_clock, wait_clock):
    nc = self.nc
    drain_inst = nc.sync.drain()
    wait_clock.add_sem_waits(
        drain_inst.ins, tile_mod.ScopedClock({None: tick_clock.global_clock})
    )
    _split_inst_waits(nc, drain_inst)
    nc.all_engine_barrier()
    assert self.sems is not None
    popped = nc._tile_sem_poison_stack.pop()
    assert popped is self._sem_poison
    nc.clear_and_free_semaphores(list(self.sems.allocated().values()))
    nc.all_engine_barrier()


_orig_commit_instruction = tile_mod.TileContext._commit_instruction
if getattr(tile_mod.TileContext, "_flame_wait_patch", False) is False:
    tile_mod.TileContext._commit_instruction = _patched_commit_instruction
    tile_mod.TileContext._drain_and_barrier = _patched_drain_and_barrier
    tile_mod.TileContext._flame_wait_patch = True

# ----------------------------------------------------------- host-side math


def _rodrigues(r):
    angle = np.linalg.norm(r, axis=-1, keepdims=True) + 1e-8
    axis = r / angle
    x, y, z = axis[..., 0], axis[..., 1], axis[..., 2]
    zero = np.zeros_like(x)
    K = np.stack([zero, -z, y, z, zero, -x, -y, x, zero], axis=-1)
    K = K.reshape(r.shape[:-1] + (3, 3))
    s = np.sin(angle)[..., None]
    c = np.cos(angle)[..., None]
    return np.eye(3, dtype=r.dtype) + s * K + (1.0 - c) * (K @ K)


def _split_hi_lo(x):
    hi = x.astype(np.float32).astype(BF16)
    lo = (x.astype(np.float32) - hi.astype(np.float32)).astype(BF16)
    return np.ascontiguousarray(hi), np.ascontiguousarray(lo)


def _plane_cn(cn):
    return cn // 4, cn % 4


def _host_batch_prep(shape, expression, rotation, neck, jaw, eyeballs,
                     v_template, shapedirs, J_regressor):
    f64 = np.float64
    b = shape.shape[0]
    betas = np.concatenate([shape, expression], axis=1).astype(f64)
    full_pose = np.concatenate([rotation, neck, jaw, eyeballs], axis=1).astype(f64)

    jr = J_regressor.astype(f64)
    J0 = jr @ v_template.astype(f64)                                   # [5,3]
    SJ = np.einsum('jv,vcl->ljc', jr, shapedirs.astype(f64)).reshape(400, NJ * 3)
    J = (J0.reshape(-1) + betas @ SJ).reshape(b, NJ, 3)

    rot_mats = _rodrigues(full_pose.reshape(b, NJ, 3))
    pose_feature = (rot_mats[:, 1:] - np.eye(3, dtype=f64)).reshape(b, 36)

    rel_joints = np.concatenate([J[:, :1], J[:, 1:] - J[:, PARENTS[1:]]], axis=1)
    T_local = np.zeros((b, NJ, 4, 4), dtype=f64)
    T_local[:, :, :3, :3] = rot_mats
    T_local[:, :, :3, 3] = rel_joints
    T_local[:, :, 3, 3] = 1.0
    chain = [T_local[:, 0]]
    for j in range(1, NJ):
        chain.append(chain[PARENTS[j]] @ T_local[:, j])
    A = np.stack(chain, axis=1)

    j_hom = np.concatenate([J, np.zeros_like(J[..., :1])], axis=-1)
    t_corr = np.einsum('bjmn,bjn->bjm', A, j_hom)
    A_rel = (A - np.concatenate(
        [np.zeros_like(A[..., :3]), t_corr[..., None]], axis=-1)
    ).astype(np.float32)                                               # [B,5,4,4]

    # betas_aug [437, B]: rows 0-399 betas.T, 400 ones, 401-436 pose_feature.T
    betas_aug = np.empty((KB, b), dtype=np.float32)
    betas_aug[:400] = betas.T
    betas_aug[400] = 1.0
    betas_aug[401:] = pose_feature.T
    bt_hi, bt_lo = _split_hi_lo(betas_aug)
    bt_stk = np.concatenate([bt_hi, bt_lo], axis=0)        # [2*KB, B] bf16

    # stacked row-tiled A_rel operand: artc[c] rows 32g+u pair with
    # wrt_stk rows so one K=15 matmul per plane (c, n=g) computes
    # w_hi*A_hi + w_hi*A_lo + w_lo*A_hi
    a_hi, a_lo = _split_hi_lo(A_rel)
    a_hi = a_hi.astype(np.float32)
    a_lo = a_lo.astype(np.float32)
    artc = np.zeros((3, 128, b), dtype=np.float32)
    for c in range(3):
        for n in range(4):
            for j in range(NJ):
                artc[c, 32 * n + j] = a_hi[:, j, c, n]
                artc[c, 32 * n + 5 + j] = a_lo[:, j, c, n]
                artc[c, 32 * n + 10 + j] = a_hi[:, j, c, n]
    artc = artc.astype(BF16)
    return bt_stk, artc


def _host_model_prep(v_template, shapedirs, posedirs, lbs_weights):
    # sdt_aug [3, 437, VP] matching betas_aug rows
    sdt = np.zeros((3, KB, VP), dtype=np.float32)
    sdt[:, :400, :V] = shapedirs.transpose(1, 2, 0)
    sdt[:, 400, :V] = v_template.T
    sdt[:, 401:, :V] = posedirs.reshape(36, V, 3).transpose(2, 0, 1)
    sdt_hi, sdt_lo = _split_hi_lo(sdt)
    sdt_stk = np.concatenate([sdt_hi, sdt_lo], axis=1)     # [3, 2*KB, VP] bf16

    # wrt_stk [128, VP]: rows 32g+u = [w_hi; w_hi; w_lo][u] (u<15),
    # identical for every group g
    w_hi, w_lo = _split_hi_lo(lbs_weights)
    w_hi = w_hi.astype(np.float32)
    w_lo = w_lo.astype(np.float32)
    wrt = np.zeros((128, VP), dtype=np.float32)
    for g in range(4):
        for j in range(NJ):
            wrt[32 * g + j, :V] = w_hi[:, j]
            wrt[32 * g + 5 + j, :V] = w_hi[:, j]
            wrt[32 * g + 10 + j, :V] = w_lo[:, j]
    wrt_stk = wrt.astype(BF16)
    return sdt_stk, wrt_stk

# ------------------------------------------------------------ device kernel


def _build_device_program():
    nc = bass.Bass("TRN2", target_bir_lowering=False, debug=False)
    f32 = mybir.dt.float32
    f32r = mybir.dt.float32r
    bf16 = mybir.dt.bfloat16

    KD = 2 * KB      # deduplicated rows in DRAM: [hi; lo]
    sdt = nc.dram_tensor("sdt", [3, KD, NVC], bf16, kind="ExternalInput").ap()
    wrt = nc.dram_tensor("wrt", [128, NVC], bf16, kind="ExternalInput").ap()
    bt = nc.dram_tensor("bt", [KD, B], bf16, kind="ExternalInput").ap()
    artc = nc.dram_tensor("artc", [3, 128, B], bf16, kind="ExternalInput").ap()
    out = nc.dram_tensor("out", [3, NVC, B], f32, kind="ExternalOutput").ap()

    def _stk_pieces(k0, kn, th):
        # stacked row r maps to dram row r if r < th else r - th
        # (bt stack [hi;lo;hi] -> th=2*KB; sdt stack [hi;hi;lo] -> th=KB)
        if k0 + kn <= th:
            return [(0, k0, kn)]
        if k0 >= th:
            return [(0, k0 - th, kn)]
        n1 = th - k0
        return [(0, k0, n1), (n1, 0, kn - n1)]

    with tile_mod.TileContext(nc) as tc, ExitStack() as ctx:
        cpool = ctx.enter_context(tc.tile_pool(name="const", bufs=1))
        spool = ctx.enter_context(tc.tile_pool(name="stream", bufs=2))
        vpool = ctx.enter_context(tc.tile_pool(name="vposed", bufs=2))
        tpool = ctx.enter_context(tc.tile_pool(name="tblend", bufs=2))
        apool = ctx.enter_context(tc.tile_pool(name="apply", bufs=2))
        ps_v = ctx.enter_context(tc.tile_pool(name="psv", bufs=2, space="PSUM"))
        ps_t = ctx.enter_context(tc.tile_pool(name="pst", bufs=4, space="PSUM"))

        # resident operands (scalar HWDGE queue so the per-chunk streaming
        # DMAs on the sync queue aren't stuck behind them at startup)
        btt = []
        for ki, (k0, kn) in enumerate(KCH):
            th = cpool.tile([kn, B], bf16, tag=f"bt{ki}", name=f"bt{ki}")
            for (toff, src0, n) in _stk_pieces(k0, kn, 2 * KB):
                nc.scalar.dma_start(th[toff:toff + n, :], bt[src0:src0 + n, :])
            btt.append(th)
        wrtt = cpool.tile([128, NVC], bf16, tag="wrtt")
        nc.scalar.dma_start(wrtt[:], wrt[:, :])
        artt = []
        for c in range(3):
            t = cpool.tile([128, B], bf16, tag=f"artc{c}", name=f"artc{c}")
            nc.scalar.dma_start(t[:], artc[c, :, :])
            artt.append(t)

        for k in range(NCHUNK):
            vs = slice(k * 128, (k + 1) * 128)

            # stream this chunk's vposed lhsT tiles (stacked bf16)
            st = []
            for c in range(3):
                row = []
                for ki, (k0, kn) in enumerate(KCH):
                    t = spool.tile([kn, 128], bf16, tag=f"st{c}_{ki}",
                                   name=f"st{c}_{ki}")
                    for (toff, src0, n) in _stk_pieces(k0, kn, KB):
                        nc.sync.dma_start(t[toff:toff + n, :],
                                          sdt[c, src0:src0 + n, vs])
                    row.append(t)
                st.append(row)

            # 1) vposed planes [128, B]: stacked hi/lo, K=1311 in 11 chunks
            nkc = len(KCH)
            vp = []
            for c in range(3):
                dst = vpool.tile([128, B], mybir.dt.float32, tag=f"vp{c}")
                for nsp in range(B // 1024):
                    acc = ps_v.tile([128, 1024], mybir.dt.float32, tag="psv")
                    for half in range(2):
                        ps = slice(half * NS, (half + 1) * NS)
                        ns = 2 * nsp + half
                        bs = slice(ns * NS, (ns + 1) * NS)
                        for ki in range(nkc):
                            nc.tensor.matmul(
                                acc[:, ps], lhsT=st[c][ki][:],
                                rhs=btt[ki][:, bs],
                                start=(ki == 0), stop=(ki == nkc - 1))
                    nc.scalar.copy(out=dst[:, nsp * 1024:(nsp + 1) * 1024],
                                   in_=acc[:])
                vp.append(dst)

            # 2+3) per (batch-half, c): row-tiled K=15 stacked-bf16 T blend
            # for the 4 planes of this c, then the affine apply on DVE
            for h in range(B // BH):
                hb = slice(h * BH, (h + 1) * BH)
                for c in range(3):
                    tt = tpool.tile([128, 4 * BH], mybir.dt.float32, tag="tt")
                    for ns in range(BH // NS):
                        src = slice(h * BH + ns * NS, h * BH + (ns + 1) * NS)
                        accs = [ps_t.tile([128, NS], mybir.dt.float32,
                                          name=f"tacc{g_}", tag="pst")
                                for g_ in range(4)]
                        for n in range(4):
                            p0 = 32 * n
                            nc.tensor.matmul(
                                accs[n][:],
                                lhsT=wrtt[p0:p0 + 15, vs],
                                rhs=artt[c][p0:p0 + 15, src],
                                start=True, stop=True,
                                tile_position=(p0, 0))
                        for n in range(4):
                            nc.scalar.copy(
                                out=tt[:, n * BH + ns * NS:n * BH + (ns + 1) * NS],
                                in_=accs[n][:])

                    def tsl(n):
                        return tt[:, n * BH:(n + 1) * BH]
                    ma = apool.tile([128, BH], mybir.dt.float32, tag="ma")
                    mb = apool.tile([128, BH], mybir.dt.float32, tag="mb")
                    nc.vector.tensor_mul(ma[:], tsl(0), vp[0][:, hb])
                    nc.vector.tensor_mul(mb[:], tsl(1), vp[1][:, hb])
                    nc.vector.tensor_add(ma[:], ma[:], mb[:])
                    nc.vector.tensor_mul(mb[:], tsl(2), vp[2][:, hb])
                    nc.vector.tensor_add(mb[:], mb[:], tsl(3))
                    # final add rides the output DMA (SWDGE accumulate)
                    nc.gpsimd.dma_start(out[c, vs, hb], ma[:])
                    nc.gpsimd.dma_start(out[c, vs, hb], mb[:],
                                        accum_op=mybir.AluOpType.add)
    return nc


_NC_CACHE = {}


def _get_nc():
    if "nc" not in _NC_CACHE:
        _NC_CACHE["nc"] = _build_device_program()
    return _NC_CACHE["nc"]

# ---------------------------------------------------------------- entry


def build_in_maps(shape, expression, rotation, neck, jaw, eyeballs,
                  v_template, shapedirs, posedirs, J_regressor, lbs_weights):
    bt_stk, artc = _host_batch_prep(
        shape, expression, rotation, neck, jaw, eyeballs,
        v_template, shapedirs, J_regressor)
    sdt_stk, wrt_stk = _host_model_prep(
        v_template, shapedirs, posedirs, lbs_weights)

    in_maps = []
    for i in range(NCORES):
        v0, v1 = i * NVC, (i + 1) * NVC
        in_maps.append({
            "sdt": np.ascontiguousarray(sdt_stk[:, :, v0:v1]),
            "wrt": np.ascontiguousarray(wrt_stk[:, v0:v1]),
            "bt": bt_stk,
            "artc": artc,
        })
    return in_maps


def kernel(shape, expression, rotation, neck, jaw, eyeballs,
           v_template, shapedirs, posedirs, J_regressor, lbs_weights):
    in_maps = build_in_maps(shape, expression, rotation, neck, jaw, eyeballs,
                            v_template, shapedirs, posedirs, J_regressor,
                            lbs_weights)
    nc = _get_nc()
    res = run_bass_kernel_spmd(nc, in_maps, core_ids=list(range(NCORES)))

    full = np.concatenate([res.results[i]["out"] for i in range(NCORES)], axis=1)
    verts = np.ascontiguousarray(full[:, :V, :].transpose(2, 1, 0))
    return verts.astype(np.float32)


# revision 29
# speedup vs baseline: 1.2609x; 1.0995x over previous
"""FLAME layer on 8 Trainium2 NeuronCores (Bass/Tile).

Strategy (vertex-parallel):
  * V=5023 padded to 5120 and split 640 vertices/core; every core handles
    the full batch B=2048 for its vertex slice. This replicates only the
    small per-batch operands (betas, pose features, joint transforms)
    while the big model tensor (shapedirs, 24MB) is sliced 1/8 per core.
  * Host (O(B) + O(model) prep): rodrigues, forward-kinematic chain,
    A_rel; J is linear in betas (J = J0 + betas @ SJ with SJ precomputed
    from J_regressor x shapedirs), so no device dependency on v_shaped.
    Host also pre-transposes operands so the device does zero transposes.
  * Device (all O(B*V) work), per 128-vertex chunk in [v(part), b(free)]
    layout:
      1. vposed_c = sdt_aug_c.T @ betas_aug  (PE; K=437 = 400 blendshape
         rows + v_template x ones row + 36 posedirs rows; the bf16 hi/lo
         3-pass is stacked into K=1311 rows [hi;hi;lo] x [hi;lo;hi] for
         ~1e-6 accuracy at bf16 speed)
      2. T = blend of per-joint transforms (PE): per output plane one
         K=15 bf16 matmul whose rows stack the hi/lo split 3-pass
         ([w_hi;w_hi;w_lo] x [A_hi;A_lo;A_hi]), row-tiled 4x via
         tile_position so the 4 planes of one c run concurrently
      3. verts_c = T_c0*vp_x + T_c1*vp_y + T_c2*vp_z + T_c3  (DVE)
      4. DMA out [3, 640, 2048]; host reassembles [B, V, 3].
"""

import os
from contextlib import ExitStack

import ml_dtypes
import numpy as np

import bass_rust
import concourse.bass as bass
import concourse.mybir as mybir
import concourse.tile as tile_mod
from concourse.bass_utils import run_bass_kernel_spmd

# ---------------------------------------------------------------- constants
B = 2048
V = 5023
VP = 5120            # padded vertex count (8 cores x 640)
NVC = VP // 8        # vertices per core
NCHUNK = NVC // 128  # 128-vertex chunks per core (5)
NJ = 5
NCORES = 8
BH = 1024            # batch half (free-dim tile for T/apply)
KB = 437             # vposed contraction rows: 400 betas + 1 ones + 36 pose
KS = 3 * KB          # stacked hi/lo rows: [hi;hi;lo] x [hi;lo;hi]
KCH = [(k, min(128, KS - k)) for k in range(0, KS, 128)]
NS = 512             # matmul free-dim slice (one PSUM bank)
PARENTS = np.array([-1, 0, 1, 1, 1])

# T-blend: per c one round of 4 concurrent row-tiled K=15 matmuls,
# group g handles plane (c, n=g).

BF16 = ml_dtypes.bfloat16

# ------------------------------------------------- walrus multi-wait patch
# This walrus build accepts only ONE sem-wait per instruction (CTRL and
# LW queue structs alike), but Tile freely assigns several. Split the
# surplus waits onto same-engine NOPs emitted immediately before the
# instruction — the engine stalls on each NOP's wait first, so the
# gating semantics are identical.


def _patched_commit_instruction(self, inst, lazy_reg_writes=True):
    si = inst.sync_info
    if si is not None and len(si.on_wait) > 1:
        waits = list(si.on_wait)
        inst.sync_info = bass_rust.SyncInfo(
            on_update=list(si.on_update), on_wait=waits[:1]
        )
        for w in waits[1:]:
            nop = mybir.InstNoOp(
                name=self.nc.get_next_instruction_name(),
                engine=inst.engine,
                ins=[],
                outs=[],
                bass_nofuse=True,
                sync_info=bass_rust.SyncInfo(on_update=[], on_wait=[w]),
            )
            _orig_commit_instruction(self, nop, lazy_reg_writes=False)
    return _orig_commit_instruction(self, inst, lazy_reg_writes)


def _split_inst_waits(nc, inst):
    si = inst.ins.sync_info
    if si is None:
        return
    waits = list(si.on_wait)
    if len(waits) <= 1:
        return
    inst.ins.sync_info = bass_rust.SyncInfo(
        on_update=list(si.on_update), on_wait=waits[:1]
    )
    for i in range(1, len(waits)):
        nop = nc.sync.nop(nofuse=True, hint="drain_wait_split")
        nop.ins.sync_info = bass_rust.SyncInfo(on_update=[], on_wait=[waits[i]])


def _patched_drain_and_barrier(self, tick_clock, wait_clock):
    nc = self.nc
    drain_inst = nc.sync.drain()
    wait_clock.add_sem_waits(
        drain_inst.ins, tile_mod.ScopedClock({None: tick_clock.global_clock})
    )
    _split_inst_waits(nc, drain_inst)
    nc.all_engine_barrier()
    assert self.sems is not None
    popped = nc._tile_sem_poison_stack.pop()
    assert popped is self._sem_poison
    nc.clear_and_free_semaphores(list(self.sems.allocated().values()))
    nc.all_engine_barrier()


_orig_commit_instruction = tile_mod.TileContext._commit_instruction
if getattr(tile_mod.TileContext, "_flame_wait_patch", False) is False:
    tile_mod.TileContext._commit_instruction = _patched_commit_instruction
    tile_mod.TileContext._drain_and_barrier = _patched_drain_and_barrier
    tile_mod.TileContext._flame_wait_patch = True

# ----------------------------------------------------------- host-side math


def _rodrigues(r):
    angle = np.linalg.norm(r, axis=-1, keepdims=True) + 1e-8
    axis = r / angle
    x, y, z = axis[..., 0], axis[..., 1], axis[..., 2]
    zero = np.zeros_like(x)
    K = np.stack([zero, -z, y, z, zero, -x, -y, x, zero], axis=-1)
    K = K.reshape(r.shape[:-1] + (3, 3))
    s = np.sin(angle)[..., None]
    c = np.cos(angle)[..., None]
    return np.eye(3, dtype=r.dtype) + s * K + (1.0 - c) * (K @ K)


def _split_hi_lo(x):
    hi = x.astype(np.float32).astype(BF16)
    lo = (x.astype(np.float32) - hi.astype(np.float32)).astype(BF16)
    return np.ascontiguousarray(hi), np.ascontiguousarray(lo)


def _plane_cn(cn):
    return cn // 4, cn % 4


def _host_batch_prep(shape, expression, rotation, neck, jaw, eyeballs,
                     v_template, shapedirs, J_regressor):
    f64 = np.float64
    b = shape.shape[0]
    betas = np.concatenate([shape, expression], axis=1).astype(f64)
    full_pose = np.concatenate([rotation, neck, jaw, eyeballs], axis=1).astype(f64)

    jr = J_regressor.astype(f64)
    J0 = jr @ v_template.astype(f64)                                   # [5,3]
    SJ = np.einsum('jv,vcl->ljc', jr, shapedirs.astype(f64)).reshape(400, NJ * 3)
    J = (J0.reshape(-1) + betas @ SJ).reshape(b, NJ, 3)

    rot_mats = _rodrigues(full_pose.reshape(b, NJ, 3))
    pose_feature = (rot_mats[:, 1:] - np.eye(3, dtype=f64)).reshape(b, 36)

    rel_joints = np.concatenate([J[:, :1], J[:, 1:] - J[:, PARENTS[1:]]], axis=1)
    T_local = np.zeros((b, NJ, 4, 4), dtype=f64)
    T_local[:, :, :3, :3] = rot_mats
    T_local[:, :, :3, 3] = rel_joints
    T_local[:, :, 3, 3] = 1.0
    chain = [T_local[:, 0]]
    for j in range(1, NJ):
        chain.append(chain[PARENTS[j]] @ T_local[:, j])
    A = np.stack(chain, axis=1)

    j_hom = np.concatenate([J, np.zeros_like(J[..., :1])], axis=-1)
    t_corr = np.einsum('bjmn,bjn->bjm', A, j_hom)
    A_rel = (A - np.concatenate(
        [np.zeros_like(A[..., :3]), t_corr[..., None]], axis=-1)
    ).astype(np.float32)                                               # [B,5,4,4]

    # betas_aug [437, B]: rows 0-399 betas.T, 400 ones, 401-436 pose_feature.T
    betas_aug = np.empty((KB, b), dtype=np.float32)
    betas_aug[:400] = betas.T
    betas_aug[400] = 1.0
    betas_aug[401:] = pose_feature.T
    bt_hi, bt_lo = _split_hi_lo(betas_aug)
    bt_stk = np.concatenate([bt_hi, bt_lo], axis=0)        # [2*KB, B] bf16

    # stacked row-tiled A_rel operand: artc[c] rows 32g+u pair with
    # wrt_stk rows so one K=15 matmul per plane (c, n=g) computes
    # w_hi*A_hi + w_hi*A_lo + w_lo*A_hi
    a_hi, a_lo = _split_hi_lo(A_rel)
    a_hi = a_hi.astype(np.float32)
    a_lo = a_lo.astype(np.float32)
    artc = np.zeros((3, 128, b), dtype=np.float32)
    for c in range(3):
        for n in range(4):
            for j in range(NJ):
                artc[c, 32 * n + j] = a_hi[:, j, c, n]
                artc[c, 32 * n + 5 + j] = a_lo[:, j, c, n]
                artc[c, 32 * n + 10 + j] = a_hi[:, j, c, n]
    artc = artc.astype(BF16)
    return bt_stk, artc


def _host_model_prep(v_template, shapedirs, posedirs, lbs_weights):
    # sdt_aug [3, 437, VP] matching betas_aug rows
    sdt = np.zeros((3, KB, VP), dtype=np.float32)
    sdt[:, :400, :V] = shapedirs.transpose(1, 2, 0)
    sdt[:, 400, :V] = v_template.T
    sdt[:, 401:, :V] = posedirs.reshape(36, V, 3).transpose(2, 0, 1)
    sdt_hi, sdt_lo = _split_hi_lo(sdt)
    sdt_stk = np.concatenate([sdt_hi, sdt_lo], axis=1)     # [3, 2*KB, VP] bf16

    # wrt_stk [128, VP]: rows 32g+u = [w_hi; w_hi; w_lo][u] (u<15),
    # identical for every group g
    w_hi, w_lo = _split_hi_lo(lbs_weights)
    w_hi = w_hi.astype(np.float32)
    w_lo = w_lo.astype(np.float32)
    wrt = np.zeros((128, VP), dtype=np.float32)
    for g in range(4):
        for j in range(NJ):
            wrt[32 * g + j, :V] = w_hi[:, j]
            wrt[32 * g + 5 + j, :V] = w_hi[:, j]
            wrt[32 * g + 10 + j, :V] = w_lo[:, j]
    wrt_stk = wrt.astype(BF16)
    return sdt_stk, wrt_stk

# ------------------------------------------------------------ device kernel


def _build_device_program():
    nc = bass.Bass("TRN2", target_bir_lowering=False, debug=False)
    f32 = mybir.dt.float32
    f32r = mybir.dt.float32r
    bf16 = mybir.dt.bfloat16

    KD = 2 * KB      # deduplicated rows in DRAM: [hi; lo]
    sdt = nc.dram_tensor("sdt", [3, KD, NVC], bf16, kind="ExternalInput").ap()
    wrt = nc.dram_tensor("wrt", [128, NVC], bf16, kind="ExternalInput").ap()
    bt = nc.dram_tensor("bt", [KD, B], bf16, kind="ExternalInput").ap()
    artc = nc.dram_tensor("artc", [3, 128, B], bf16, kind="ExternalInput").ap()
    out = nc.dram_tensor("out", [3, NVC, B], f32, kind="ExternalOutput").ap()

    def _stk_pieces(k0, kn, th):
        # stacked row r maps to dram row r if r < th else r - th
        # (bt stack [hi;lo;hi] -> th=2*KB; sdt stack [hi;hi;lo] -> th=KB)
        if k0 + kn <= th:
            return [(0, k0, kn)]
        if k0 >= th:
            return [(0, k0 - th, kn)]
        n1 = th - k0
        return [(0, k0, n1), (n1, 0, kn - n1)]

    with tile_mod.TileContext(nc) as tc, ExitStack() as ctx:
        cpool = ctx.enter_context(tc.tile_pool(name="const", bufs=1))
        spool = ctx.enter_context(tc.tile_pool(name="stream", bufs=2))
        vpool = ctx.enter_context(tc.tile_pool(name="vposed", bufs=2))
        tpool = ctx.enter_context(tc.tile_pool(name="tblend", bufs=2))
        apool = ctx.enter_context(tc.tile_pool(name="apply", bufs=2))
        ps_v = ctx.enter_context(tc.tile_pool(name="psv", bufs=2, space="PSUM"))
        ps_t = ctx.enter_context(tc.tile_pool(name="pst", bufs=4, space="PSUM"))

        # resident operands (scalar HWDGE queue so the per-chunk streaming
        # DMAs on the sync queue aren't stuck behind them at startup)
        btt = []
        for ki, (k0, kn) in enumerate(KCH):
            th = cpool.tile([kn, B], bf16, tag=f"bt{ki}", name=f"bt{ki}")
            for (toff, src0, n) in _stk_pieces(k0, kn, 2 * KB):
                nc.scalar.dma_start(th[toff:toff + n, :], bt[src0:src0 + n, :])
            btt.append(th)
        wrtt = cpool.tile([128, NVC], bf16, tag="wrtt")
        nc.scalar.dma_start(wrtt[:], wrt[:, :])
        artt = []
        for c in range(3):
            t = cpool.tile([128, B], bf16, tag=f"artc{c}", name=f"artc{c}")
            nc.scalar.dma_start(t[:], artc[c, :, :])
            artt.append(t)

        for k in range(NCHUNK):
            vs = slice(k * 128, (k + 1) * 128)

            # stream this chunk's vposed lhsT tiles (stacked bf16)
            st = []
            for c in range(3):
                row = []
                for ki, (k0, kn) in enumerate(KCH):
                    t = spool.tile([kn, 128], bf16, tag=f"st{c}_{ki}",
                                   name=f"st{c}_{ki}")
                    for (toff, src0, n) in _stk_pieces(k0, kn, KB):
                        nc.sync.dma_start(t[toff:toff + n, :],
                                          sdt[c, src0:src0 + n, vs])
                    row.append(t)
                st.append(row)

            # 1) vposed planes [128, B]: stacked hi/lo, K=1311 in 11 chunks
            nkc = len(KCH)
            vp = []
            for c in range(3):
                dst = vpool.tile([128, B], mybir.dt.float32, tag=f"vp{c}")
                for nsp in range(B // 1024):
                    acc = ps_v.tile([128, 1024], mybir.dt.float32, tag="psv")
                    for half in range(2):
                        ps = slice(half * NS, (half + 1) * NS)
                        ns = 2 * nsp + half
                        bs = slice(ns * NS, (ns + 1) * NS)
                        for ki in range(nkc):
                            nc.tensor.matmul(
                                acc[:, ps], lhsT=st[c][ki][:],
                                rhs=btt[ki][:, bs],
                                start=(ki == 0), stop=(ki == nkc - 1))
                    nc.scalar.copy(out=dst[:, nsp * 1024:(nsp + 1) * 1024],
                                   in_=acc[:])
                vp.append(dst)

            # 2+3) per (batch-half, c): row-tiled K=15 stacked-bf16 T blend
            # for the 4 planes of this c, then the affine apply on DVE
            for h in range(B // BH):
                hb = slice(h * BH, (h + 1) * BH)
                for c in range(3):
                    tt = tpool.tile([128, 4 * BH], mybir.dt.float32, tag="tt")
                    for ns in range(BH // NS):
                        src = slice(h * BH + ns * NS, h * BH + (ns + 1) * NS)
                        accs = [ps_t.tile([128, NS], mybir.dt.float32,
                                          name=f"tacc{g_}", tag="pst")
                                for g_ in range(4)]
                        for n in range(4):
                            p0 = 32 * n
                            nc.tensor.matmul(
                                accs[n][:],
                                lhsT=wrtt[p0:p0 + 15, vs],
                                rhs=artt[c][p0:p0 + 15, src],
                                start=True, stop=True,
                                tile_position=(p0, 0))
                        for n in range(4):
                            nc.scalar.copy(
                                out=tt[:, n * BH + ns * NS:n * BH + (ns + 1) * NS],
                                in_=accs[n][:])

                    def tsl(n):
                        return tt[:, n * BH:(n + 1) * BH]
                    ma = apool.tile([128, BH], mybir.dt.float32, tag="ma")
                    mb = apool.tile([128, BH], mybir.dt.float32, tag="mb")
                    nc.vector.tensor_mul(ma[:], tsl(0), vp[0][:, hb])
                    nc.vector.tensor_mul(mb[:], tsl(1), vp[1][:, hb])
                    nc.vector.tensor_add(ma[:], ma[:], mb[:])
                    nc.vector.tensor_mul(mb[:], tsl(2), vp[2][:, hb])
                    nc.vector.tensor_add(mb[:], mb[:], tsl(3))
                    if k < NCHUNK - 1:
                        # final add rides the output DMA (SWDGE accumulate)
                        nc.gpsimd.dma_start(out[c, vs, hb], ma[:])
                        nc.gpsimd.dma_start(out[c, vs, hb], mb[:],
                                            accum_op=mybir.AluOpType.add)
                    else:
                        # tail chunk: keep the DMA path short
                        nc.vector.tensor_add(ma[:], ma[:], mb[:])
                        nc.sync.dma_start(out[c, vs, hb], ma[:])
    return nc


_NC_CACHE = {}


def _get_nc():
    if "nc" not in _NC_CACHE:
        _NC_CACHE["nc"] = _build_device_program()
    return _NC_CACHE["nc"]

# ---------------------------------------------------------------- entry


def build_in_maps(shape, expression, rotation, neck, jaw, eyeballs,
                  v_template, shapedirs, posedirs, J_regressor, lbs_weights):
    bt_stk, artc = _host_batch_prep(
        shape, expression, rotation, neck, jaw, eyeballs,
        v_template, shapedirs, J_regressor)
    sdt_stk, wrt_stk = _host_model_prep(
        v_template, shapedirs, posedirs, lbs_weights)

    in_maps = []
    for i in range(NCORES):
        v0, v1 = i * NVC, (i + 1) * NVC
        in_maps.append({
            "sdt": np.ascontiguousarray(sdt_stk[:, :, v0:v1]),
            "wrt": np.ascontiguousarray(wrt_stk[:, v0:v1]),
            "bt": bt_stk,
            "artc": artc,
        })
    return in_maps


def kernel(shape, expression, rotation, neck, jaw, eyeballs,
           v_template, shapedirs, posedirs, J_regressor, lbs_weights):
    in_maps = build_in_maps(shape, expression, rotation, neck, jaw, eyeballs,
                            v_template, shapedirs, posedirs, J_regressor,
                            lbs_weights)
    nc = _get_nc()
    res = run_bass_kernel_spmd(nc, in_maps, core_ids=list(range(NCORES)))

    full = np.concatenate([res.results[i]["out"] for i in range(NCORES)], axis=1)
    verts = np.ascontiguousarray(full[:, :V, :].transpose(2, 1, 0))
    return verts.astype(np.float32)
